# revision 1
# baseline (speedup 1.0000x reference)
"""Linear-chain CRF Viterbi decode on Trainium2 (Bass/Tile), 8-core data parallel.

Algorithm (exact bitwise match to the f32 jax reference):
  forward:  alpha_0 = emit_0;  alpha_t[j] = max_i(alpha_{t-1}[i] + T[i,j]) + emit_t[j]
            (scores materialized with single f32 adds, max via tensor_reduce ->
             bitwise-identical to the reference's `alpha[:,None]+T` + max)
            alphas for every t are spilled to a DRAM scratch buffer.
  backward: tag_T = first-argmax(alpha_T);
            tag_t = first-argmax_i(alpha_t[i] + T[i, tag_{t+1}]).
            The gathered transitions column T[:, tag_{t+1}] is produced per batch
            row with a one-hot matmul on the PE (h -> PE-transpose -> block-diag
            matmul), which is exact (products with 0.0/1.0).  First-argmax is
            enforced with a reset-cumsum scan so exact score ties resolve to the
            lowest index, matching jnp.argmax.
  output:   one-hot rows written as f32.

Sharding: pure batch data-parallelism, batch 8192 -> 8 cores x 1024 rows.
"""

import numpy as np
from contextlib import ExitStack

B = 8192
T = 512
K = 24
NCORES = 8
BL = B // NCORES          # rows per core
P = 128                   # partitions

_prog_cache = {}


def _bview(sl, dims):
    """AP with custom free dims (incl. step-0 broadcast), keeping the slice's
    offset and partition pair."""
    from concourse.ap import AP
    return AP(sl.tensor, sl.offset, [list(sl.ap[0])] + [list(d) for d in dims])


def _dview(ap, offset, dims):
    """Arbitrary strided view of a DRAM tensor ([[step,count],...], elem offset)."""
    from concourse.ap import AP
    return AP(ap.tensor, offset, [list(d) for d in dims])


def _split_excess_waits(nc):
    """Walrus rejects engine instructions whose embedded sync struct carries
    more than one semaphore wait ("Too many sync wait commands").  Engine
    instruction streams execute in order, so moving excess waits onto
    ENGINE_NOPs inserted immediately before the instruction is semantically
    identical.  Sequencer-only instructions (SP/DMA) use standalone wait
    commands and are exempt."""
    from concourse import mybir
    nop_opcode = nc.isa.Opcode.NEURON_ISA_TPB_OPCODE_ENGINE_NOP
    eng_map = {
        mybir.EngineType.DVE: nc.vector,
        mybir.EngineType.Activation: nc.scalar,
        mybir.EngineType.PE: nc.tensor,
        mybir.EngineType.Pool: nc.gpsimd,
    }
    ctr = 0
    for f in nc.m.functions:
        for blk in f.blocks:
            changed = False
            out = []
            for ins in blk.instructions:
                si = ins.sync_info
                if si is not None and len(si.on_wait) > 1:
                    for w in si.on_wait:
                        if ins.engine in eng_map:
                            # Drain = benign sequencer-side stall, accepted as
                            # a wait carrier on every compute engine.
                            nop = mybir.InstDrain(name=f"WD-{ctr}", ins=[],
                                                  outs=[])
                        else:
                            nop = mybir.InstNoOp(name=f"WS-{ctr}", ins=[],
                                                 outs=[])
                        ctr += 1
                        nop.engine = ins.engine
                        nop.sync_info = mybir.SyncInfo(on_wait=[w],
                                                       on_update=[])
                        out.append(nop)
                    ins.sync_info = mybir.SyncInfo(
                        on_wait=[], on_update=list(si.on_update))
                    changed = True
                out.append(ins)
            if changed:
                blk.instructions = out
    return ctr


def build_program(BLc, Tc, CH, nd, split_waits=True, repeat=1, phases="both"):
    """Build the per-core Bass program.

    BLc: local batch rows; Tc: sequence length; CH: time-chunk size;
    nd: number of 128-row tiles whose forward score-adds run on DVE
        (the rest run on GPSIMD).
    """
    import concourse.bass as bass
    import concourse.tile as tile
    from concourse import mybir

    f32 = mybir.dt.float32
    Alu = mybir.AluOpType
    X = mybir.AxisListType.X

    NT = BLc // P             # batch tiles per core
    NCH = Tc // CH            # time chunks
    CK = CH * K               # free elems per (tile, chunk)
    # traceback PE transpose groups (<=4 tiles -> <=96 partitions each)
    groups = []
    s0 = 0
    while s0 < NT:
        g = min(4, NT - s0)
        groups.append((s0, g))
        s0 += g
    GK = groups[0][1] * K     # group free width (all groups equal for NT in {2,4,8})
    assert all(g * K == GK for _, g in groups)

    nc = bass.Bass("TRN2", target_bir_lowering=False, debug=False)

    inp = nc.dram_tensor("inp", [BLc, Tc, K], f32, kind="ExternalInput").ap()
    tb_d = nc.dram_tensor("tbcast", [P, K * K], f32, kind="ExternalInput").ap()
    bd_d = nc.dram_tensor("blockdiag", [GK, GK], f32, kind="ExternalInput").ap()
    idn_d = nc.dram_tensor("idn", [P, P], f32, kind="ExternalInput").ap()
    rp_d = nc.dram_tensor("resetpat", [P, NT * K], f32, kind="ExternalInput").ap()
    outp = nc.dram_tensor("outp", [BLc, Tc, K], f32, kind="ExternalOutput").ap()
    adram = nc.dram_tensor("alpha_scr", [NT, P, Tc, K], f32, kind="Internal").ap()

    ng = NT - nd              # gpsimd-add tiles

    with tile.TileContext(nc) as tc, ExitStack() as ctx:
        const = ctx.enter_context(tc.tile_pool(name="const", bufs=1))

        tb = const.tile([P, K * K], f32)
        nc.sync.dma_start(tb[:, :], tb_d)
        bd = const.tile([GK, GK], f32)
        nc.sync.dma_start(bd[:, :], bd_d)
        idn = const.tile([P, P], f32)
        nc.sync.dma_start(idn[:, :], idn_d)
        rp = const.tile([P, NT * K], f32)
        nc.sync.dma_start(rp[:, :], rp_d)
        if NT - nd > 0:
            # GPSIMD-local copy of tb (Pool instructions only support a single
            # sync wait, so GPSIMD reads must not depend on the DMA directly).
            tb_g = const.tile([P, K * K], f32)
            nc.gpsimd.tensor_copy(tb_g[:, :], tb[:, :])

        for _rep in range(repeat):
            if _rep:
                tc.strict_bb_all_engine_barrier()
            # ---------------- forward ----------------
            if phases in ("both", "fwd"):
              fctx = ctx.enter_context(ExitStack())
              femis = fctx.enter_context(tc.tile_pool(name="femis", bufs=2))
              fhist = fctx.enter_context(tc.tile_pool(name="fhist", bufs=2))
              fs = fctx.enter_context(tc.tile_pool(name="fs", bufs=2))
              fm = fctx.enter_context(tc.tile_pool(name="fm", bufs=2))
              prev_hist = None
              for ch in range(NCH):
                  emis = femis.tile([P, NT * CK], f32, tag="emis")
                  nc.sync.dma_start(
                      emis[:, :].rearrange("p (n c) -> p n c", c=CK),
                      _dview(inp, ch * CK,
                             [[Tc * K, P], [P * Tc * K, NT], [1, CK]]))
                  hist = fhist.tile([P, NT * CK], f32, tag="hist")
                  for t_c in range(CH):
                      t = ch * CH + t_c
                      h_sl = _bview(hist[:, t_c * K:t_c * K + 1], [[CK, NT], [1, K]])
                      e_sl = _bview(emis[:, t_c * K:t_c * K + 1], [[CK, NT], [1, K]])
                      if t == 0:
                          nc.vector.tensor_copy(h_sl, e_sl)
                          continue
                      if t_c == 0:
                          ap_base = prev_hist[:, (CH - 1) * K:(CH - 1) * K + 1]
                      else:
                          ap_base = hist[:, (t_c - 1) * K:(t_c - 1) * K + 1]
                      # scores: s[p, n, j, i] = T[i, j] + alpha[p, n, i]
                      if nd > 0:
                          s_d = fs.tile([P, nd * K * K], f32, tag="sd")
                          nc.vector.tensor_tensor(
                              _bview(s_d[:, 0:1], [[K * K, nd], [K, K], [1, K]]),
                              _bview(tb[:, 0:1], [[0, nd], [K, K], [1, K]]),
                              _bview(ap_base, [[CK, nd], [0, K], [1, K]]),
                              op=Alu.add)
                      if ng > 0:
                          s_g = fs.tile([P, ng * K * K], f32, tag="sg")
                          ap_g = _bview(
                              prev_hist[:, nd * CK + (CH - 1) * K: nd * CK + (CH - 1) * K + 1]
                              if t_c == 0 else
                              hist[:, nd * CK + (t_c - 1) * K: nd * CK + (t_c - 1) * K + 1],
                              [[CK, ng], [0, K], [1, K]])
                          nc.gpsimd.tensor_tensor(
                              _bview(s_g[:, 0:1], [[K * K, ng], [K, K], [1, K]]),
                              _bview(tb_g[:, 0:1], [[0, ng], [K, K], [1, K]]),
                              ap_g,
                              op=Alu.add)
                      m = fm.tile([P, NT * K], f32, tag="m")
                      if nd > 0:
                          nc.vector.reduce_max(
                              _bview(m[:, 0:1], [[K, nd], [1, K]]),
                              _bview(s_d[:, 0:1], [[K * K, nd], [K, K], [1, K]]),
                              axis=X)
                      if ng > 0:
                          nc.vector.reduce_max(
                              _bview(m[:, nd * K:nd * K + 1], [[K, ng], [1, K]]),
                              _bview(s_g[:, 0:1], [[K * K, ng], [K, K], [1, K]]),
                              axis=X)
                          # Scrub (after the reduce consumed s_g): a 1-elem DVE write
                          # makes the buffer's last-writer DVE, so the next GPSIMD
                          # add's WAW dep is a DVE wait (Pool instructions only
                          # support a single sync wait).
                          nc.vector.tensor_copy(s_g[:, 0:1], s_g[:, 1:2])
                      nc.vector.tensor_tensor(
                          h_sl, _bview(m[:, 0:1], [[K, NT], [1, K]]), e_sl, op=Alu.add)
                  nc.sync.dma_start(
                      _dview(adram, ch * CK,
                             [[Tc * K, P], [P * Tc * K, NT], [1, CK]]),
                      hist[:, :].rearrange("p (n c) -> p n c", c=CK))
                  prev_hist = hist
              fctx.close()
            tc.strict_bb_all_engine_barrier()

            # ---------------- backward (traceback) ----------------
            if phases in ("both", "bwd"):
              bctx = ctx.enter_context(ExitStack())
              ta = bctx.enter_context(tc.tile_pool(name="ta", bufs=2))
              to = bctx.enter_context(tc.tile_pool(name="to", bufs=2))
              tsm = bctx.enter_context(tc.tile_pool(name="tsm", bufs=3))
              th = bctx.enter_context(tc.tile_pool(name="th", bufs=3))
              tps = bctx.enter_context(tc.tile_pool(name="tps", bufs=2, space="PSUM"))
              # Two independent per-group chains: their 8-deep dependency
              # chains (PE transpose -> ACT copy -> PE matmul -> DVE x5)
              # interleave, hiding most of the per-step latency.
              h_list = [None] * len(groups)
              for ch in range(NCH - 1, -1, -1):
                  ach = ta.tile([P, NT * CK], f32, tag="ach")
                  nc.sync.dma_start(
                      ach[:, :].rearrange("p (n c) -> p n c", c=CK),
                      _dview(adram, ch * CK,
                             [[Tc * K, P], [P * Tc * K, NT], [1, CK]]))
                  och = to.tile([P, NT * CK], f32, tag="och")
                  for t_c in range(CH - 1, -1, -1):
                      t = ch * CH + t_c
                      for gi, (g0, gn) in enumerate(groups):
                          GW = gn * K
                          a_sl = _bview(
                              ach[:, g0 * CK + t_c * K:g0 * CK + t_c * K + 1],
                              [[CK, gn], [1, K]])
                          if t == Tc - 1:
                              s_v = a_sl
                          else:
                              # gather g[p, n, i] = T[i, tag_{t+1}[p, n]]
                              htp = tps.tile([GW, P], f32, tag=f"htp{gi}")
                              nc.tensor.transpose(
                                  htp[:, :], h_list[gi][:, :], idn[:, :])
                              hts = tsm.tile([GW, P], f32, tag=f"hts{gi}")
                              nc.scalar.copy(hts[:, :], htp[:, :])
                              gp = tps.tile([P, GW], f32, tag=f"gp{gi}")
                              nc.tensor.matmul(gp[:, :], hts[:, :], bd[:, :],
                                               start=True, stop=True)
                              s_t = tsm.tile([P, GW], f32, tag=f"s{gi}")
                              nc.vector.tensor_tensor(
                                  s_t[:, :], gp[:, :], a_sl, op=Alu.add)
                              s_v = _bview(s_t[:, 0:1], [[K, gn], [1, K]])
                          mx = tsm.tile([P, gn], f32, tag=f"mx{gi}")
                          nc.vector.reduce_max(
                              _bview(mx[:, 0:1], [[1, gn]]), s_v, axis=X)
                          eq = tsm.tile([P, GW], f32, tag=f"eq{gi}")
                          nc.vector.tensor_tensor(
                              _bview(eq[:, 0:1], [[K, gn], [1, K]]),
                              s_v,
                              _bview(mx[:, 0:1], [[1, gn], [0, K]]),
                              op=Alu.is_equal)
                          c = tsm.tile([P, GW], f32, tag=f"c{gi}")
                          nc.vector.tensor_tensor_scan(
                              c[:, :], rp[:, 0:GW], eq[:, :], initial=0.0,
                              op0=Alu.mult, op1=Alu.add)
                          h_new = th.tile([P, GW], f32, tag=f"h{gi}")
                          nc.vector.scalar_tensor_tensor(
                              h_new[:, :], c[:, :], 1.0, eq[:, :],
                              op0=Alu.is_equal, op1=Alu.mult)
                          h_list[gi] = h_new
                          nc.scalar.copy(
                              _bview(och[:, g0 * CK + t_c * K:
                                         g0 * CK + t_c * K + 1],
                                     [[CK, gn], [1, K]]),
                              _bview(h_new[:, 0:1], [[K, gn], [1, K]]))
                  nc.sync.dma_start(
                      _dview(outp, ch * CK,
                             [[Tc * K, P], [P * Tc * K, NT], [1, CK]]),
                      och[:, :].rearrange("p (n c) -> p n c", c=CK))
              bctx.close()
    if split_waits:
        _split_excess_waits(nc)
    return nc


def make_aux(transitions, BLc):
    """Host-side constant tensors derived from the transitions matrix."""
    NT = BLc // P
    groups = []
    s0 = 0
    while s0 < NT:
        g = min(4, NT - s0)
        groups.append((s0, g))
        s0 += g
    gn = groups[0][1]
    GK = gn * K
    Tm = np.asarray(transitions, dtype=np.float32)
    tb = np.ascontiguousarray(
        np.broadcast_to(Tm.T.reshape(1, K * K), (P, K * K))).astype(np.float32)
    bdm = np.zeros((GK, GK), np.float32)
    for g in range(gn):
        bdm[g * K:(g + 1) * K, g * K:(g + 1) * K] = Tm.T
    idn = np.eye(P, dtype=np.float32)
    rp = np.ones((P, NT * K), np.float32)
    rp[:, ::K] = 0.0
    return {"tbcast": tb, "blockdiag": bdm, "idn": idn, "resetpat": rp}


def run(inputs, transitions, trace=False, **spmd_kwargs):
    from concourse.bass_utils import run_bass_kernel_spmd

    key = (BL, T)
    if key not in _prog_cache:
        _prog_cache[key] = build_program(BL, T, CH=32, nd=NT_TILES)
    nc = _prog_cache[key]

    inputs = np.asarray(inputs, dtype=np.float32)
    aux = make_aux(transitions, BL)
    in_maps = [
        {"inp": np.ascontiguousarray(inputs[c * BL:(c + 1) * BL]), **aux}
        for c in range(NCORES)
    ]
    res = run_bass_kernel_spmd(nc, in_maps, core_ids=list(range(NCORES)),
                               trace=trace, **spmd_kwargs)
    out = np.concatenate([r["outp"] for r in res.results], axis=0)
    return np.ascontiguousarray(out, dtype=np.float32), res


NT_TILES = BL // P


def kernel(inputs, transitions):
    out, _ = run(inputs, transitions)
    return out



# revision 2
# speedup vs baseline: 1.8351x; 1.8351x over previous
"""Linear-chain CRF Viterbi decode on Trainium2 (Bass/Tile), 8-core data
parallel — fused custom-DVE implementation.

Algorithm (bitwise-exact match to the f32 jax reference):
  forward:  alpha_0 = emit_0;  alpha_t[j] = max_i(alpha_{t-1}[i] + T[i,j]) + emit_t[j]
            Per batch tile, ONE custom DVE op (SEG_MAXPLUS) streams
            T^T[(j,i)] + alpha[i] and keeps a running max that RESETS at
            each j-segment boundary — the per-j max lands at i=23.  A single
            192-wide tensor_tensor adds emissions.  All alphas spill to a
            DRAM scratch buffer (DMA-hidden).
  backward: tag_T = first-argmax(alpha_T);
            tag_t = first-argmax_i(alpha_t[i] + T[i, tag_{t+1}]).
            The T[:, tag] column is gathered with a one-hot PE matmul
            (exact: products with 0.0/1.0).  SEG_MAXPLUS computes the
            running max r of (gather + alpha); SEG_ARGMAX_ONEHOT emits the
            one-hot of the FIRST position where r equals the segment max
            (ties resolve to the lowest index, matching jnp.argmax).
            Two independent 4-tile chains interleave to hide latency.
  output:   one-hot rows written as f32.

Custom DVE ops are registered at import: plain-Python uop programs shipped
in the per-NEFF DVE table (dve_table_for_ops); the segment-reset semantics
add one FSM step-state firing on SUB_DIM_DONE.  Excess semaphore waits ride
EventSemaphore carriers (2 waits each, sequencer-level).

Sharding: pure batch data-parallelism, 8192 rows -> 8 cores x 1024 rows.
"""

import numpy as np
from contextlib import ExitStack

B = 8192
T = 512
K = 24
NCORES = 8
BL = B // NCORES          # rows per core
P = 128                   # partitions

_prog_cache = {}

# --------------------------------------------------------------------------
# Custom DVE ops: segmented (per-24) folds with reset at segment boundaries.
# --------------------------------------------------------------------------

_OPS_REGISTERED = {}


def _register_ops():
    if _OPS_REGISTERED:
        return _OPS_REGISTERED

    import concourse.dve_spec as DS
    import concourse.dve_ops as DO
    from concourse.dve_spec import Spec, Src0, Src1, One, scan, eq, AluOp
    from concourse.dve_uop import DveOpSpec, Trigger

    def _lower_segreset(spec, ver):
        """lower() plus a step-state that re-seeds every scan from its init
        at SUB_DIM_DONE (segment boundary) for exactly one element."""
        n_lanes, n_stages = DS.N_LANES[ver], DS.N_STAGES[ver]
        DS._validate_body(spec, ver)
        spec = DS._hoist_stream_invariant_ops(spec)
        scans = DS._collect(spec.body, DS.Scan)
        latches = DS._collect(spec.body, DS.Latch)
        assert scans and not latches and spec.accum is None
        placement = DS._build_placement(spec, scans, n_stages, n_lanes)
        states = list(DS._build_state_machine(spec, scans, latches, placement))
        assert len(states) == 2  # [seed, steady]
        step_ov = {}
        for s in scans:
            d = placement.node_stage[s]
            step_ov[d] = DS._Stage(s.op, DS._scan_init(s), s.expr)
        body_lvs = DS._body_scan_leaves(spec)
        consume = (Src0 in body_lvs, Src1 in body_lvs)
        st = states[1]
        states[1] = DS._State(
            placement=st.placement, consume=st.consume, overrides=st.overrides,
            trigger=(Trigger.SRC_TENSOR_DONE, Trigger.SUB_DIM_DONE,
                     Trigger.NONE),
            next=(0, 2, 0))
        states.append(DS._State(
            placement=placement, consume=consume, overrides=step_ov,
            trigger=(Trigger.SRC_TENSOR_DONE, Trigger.SUB_DIM_DONE,
                     Trigger.COUNT),
            next=(0, 2, 1), repeat=1))
        out = [DS._assemble(s) for s in states]
        for u in out:
            u.validate(ver)
        return out

    class SegDveOp(DO.DveOp):
        def compile(self, ver):
            key = (self.name, ver)
            if (r := DO._COMPILE_CACHE.get(key)) is not None:
                return r
            result = DveOpSpec(
                name=self.name,
                opcode=DO.get_dve_sub_opcode(self.name),
                uops=_lower_segreset(self.spec, ver),
                rd1_en=DS._has_src1(self.spec),
            )
            DO._COMPILE_CACHE[key] = result
            return result

    def _ref_seg_maxplus(in0, in1, c0, c1, c2):
        s = (in0 + in1).astype(np.float32)
        out = np.empty_like(s)
        run = None
        for k in range(s.shape[-1]):
            run = s[..., k] if run is None else np.maximum(run, s[..., k])
            out[..., k] = run
        return out

    def _ref_seg_max1(in0, c0, c1, c2):
        return _ref_seg_maxplus(in0, np.zeros_like(in0), c0, c1, c2)

    def _ref_seg_argmax_onehot(in0, in1, c0, c1, c2):
        e = (in0 == in1).astype(np.float32)
        c = np.cumsum(e, axis=-1)
        return (e * (c == 1.0)).astype(np.float32)

    def _mk(name, spec):
        existing = [op for op in DO.OPS if op.name == name]
        if existing:
            return existing[0]
        op = SegDveOp(name=name, spec=spec, subdim=True, uops_sha={})
        DO.OPS.append(op)
        row = DO._CUSTOM_DVE_ROW_BASE + len(DO.OPS) - 1
        assert row < 0x20, "custom-DVE opcode row overflow"
        DO._SUB_OPCODE_FOR_NAME[name] = row
        return op

    _e = eq(Src0, Src1)
    _OPS_REGISTERED.update(
        SEG_MAXPLUS=_mk("SEG_MAXPLUS",
                        Spec(body=scan(AluOp.MAX, Src0 + Src1),
                             reference=_ref_seg_maxplus)),
        SEG_MAX1=_mk("SEG_MAX1",
                     Spec(body=scan(AluOp.MAX, Src0),
                          reference=_ref_seg_max1)),
        SEG_ARGMAX_ONEHOT=_mk("SEG_ARGMAX_ONEHOT",
                              Spec(body=_e * eq(scan(AluOp.ADD, _e), One),
                                   reference=_ref_seg_argmax_onehot)),
    )
    return _OPS_REGISTERED


# --------------------------------------------------------------------------
# AP helpers
# --------------------------------------------------------------------------

def _bview(sl, dims):
    """AP with custom free dims (incl. step-0 broadcast), keeping the slice's
    offset and partition pair."""
    from concourse.ap import AP
    return AP(sl.tensor, sl.offset, [list(sl.ap[0])] + [list(d) for d in dims])


def _dview(ap, offset, dims):
    """Arbitrary strided view of a DRAM tensor ([[step,count],...], offset)."""
    from concourse.ap import AP
    return AP(ap.tensor, offset, [list(d) for d in dims])


def _split_excess_waits(nc):
    """Walrus allows at most one semaphore wait per instruction (two on
    InstEventSemaphore).  Move excess waits onto EventSemaphore carriers
    (sequencer-level, no pipeline flush) inserted immediately before the
    instruction — engine streams execute in order, so this is semantically
    identical."""
    from concourse import mybir
    ctr = 0
    for f in nc.m.functions:
        for blk in f.blocks:
            changed = False
            out = []
            for ins in blk.instructions:
                si = ins.sync_info
                if si is not None and len(si.on_wait) > 1:
                    excess = list(si.on_wait[:-1])
                    keep = si.on_wait[-1]
                    while excess:
                        pair, excess = excess[:2], excess[2:]
                        ev = mybir.InstEventSemaphore(
                            name=f"EW-{ctr}", ins=[], outs=[])
                        ctr += 1
                        ev.engine = ins.engine
                        ev.sync_info = mybir.SyncInfo(on_wait=pair,
                                                      on_update=[])
                        out.append(ev)
                    ins.sync_info = mybir.SyncInfo(
                        on_wait=[keep], on_update=list(si.on_update))
                    changed = True
                out.append(ins)
            if changed:
                blk.instructions = out
    return ctr


# --------------------------------------------------------------------------
# Program builder
# --------------------------------------------------------------------------

def build_program(BLc, Tc, CH, split_waits=True, repeat=1, phases="both"):
    import concourse.bass as bass
    import concourse.tile as tile
    from concourse import mybir

    ops = _register_ops()
    SEG_MAXPLUS = ops["SEG_MAXPLUS"]
    SEG_MAX1 = ops["SEG_MAX1"]
    SEG_ARGMAX_ONEHOT = ops["SEG_ARGMAX_ONEHOT"]

    f32 = mybir.dt.float32
    Alu = mybir.AluOpType

    NT = BLc // P             # batch tiles per core
    NCH = Tc // CH            # time chunks
    CK = CH * K               # free elems per (tile, chunk)
    KK = K * K
    groups = []
    s0 = 0
    while s0 < NT:
        g = min(4, NT - s0)
        groups.append((s0, g))
        s0 += g
    GK = max(g for _, g in groups) * K

    nc = bass.Bass("TRN2", target_bir_lowering=False, debug=False)

    inp = nc.dram_tensor("inp", [BLc, Tc, K], f32, kind="ExternalInput").ap()
    tb_d = nc.dram_tensor("tbcast", [P, KK], f32, kind="ExternalInput").ap()
    bd_d = nc.dram_tensor("blockdiag", [GK, GK], f32, kind="ExternalInput").ap()
    idn_d = nc.dram_tensor("idn", [P, P], f32, kind="ExternalInput").ap()
    outp = nc.dram_tensor("outp", [BLc, Tc, K], f32, kind="ExternalOutput").ap()
    adram = nc.dram_tensor("alpha_scr", [NT, P, Tc, K], f32, kind="Internal").ap()

    with tile.TileContext(nc) as tc, ExitStack() as ctx:
        const = ctx.enter_context(tc.tile_pool(name="const", bufs=1))

        tb = const.tile([P, KK], f32)
        nc.sync.dma_start(tb[:, :], tb_d)
        bd = const.tile([GK, GK], f32)
        nc.sync.dma_start(bd[:, :], bd_d)
        idn = const.tile([P, P], f32)
        nc.sync.dma_start(idn[:, :], idn_d)

        for _rep in range(repeat):
            if _rep:
                tc.strict_bb_all_engine_barrier()
            # ---------------- forward ----------------
            if phases in ("both", "fwd"):
              fctx = ctx.enter_context(ExitStack())
              femis = fctx.enter_context(tc.tile_pool(name="femis", bufs=2))
              fhist = fctx.enter_context(tc.tile_pool(name="fhist", bufs=2))
              fs = fctx.enter_context(tc.tile_pool(name="fs", bufs=2))
              prev_hist = None
              for ch in range(NCH):
                  emis = femis.tile([P, NT * CK], f32, tag="emis")
                  nc.sync.dma_start(
                      emis[:, :].rearrange("p (n c) -> p n c", c=CK),
                      _dview(inp, ch * CK,
                             [[Tc * K, P], [P * Tc * K, NT], [1, CK]]))
                  hist = fhist.tile([P, NT * CK], f32, tag="hist")
                  for t_c in range(CH):
                      t = ch * CH + t_c
                      h_sl = _bview(hist[:, t_c * K:t_c * K + 1], [[CK, NT], [1, K]])
                      e_sl = _bview(emis[:, t_c * K:t_c * K + 1], [[CK, NT], [1, K]])
                      if t == 0:
                          nc.vector.tensor_copy(h_sl, e_sl)
                          continue
                      if t_c == 0:
                          a_base, a_off = prev_hist, (CH - 1) * K
                      else:
                          a_base, a_off = hist, (t_c - 1) * K
                      s_all = fs.tile([P, NT * KK], f32, tag="sall")
                      for n in range(NT):
                          nc.vector._custom_dve(
                              SEG_MAXPLUS,
                              out=_bview(s_all[:, n * KK:n * KK + 1],
                                         [[K, K], [1, K]]),
                              in0=_bview(tb[:, 0:1], [[K, K], [1, K]]),
                              in1=_bview(a_base[:, n * CK + a_off:
                                                n * CK + a_off + 1],
                                         [[0, K], [1, K]]),
                          )
                      # alpha_t[n,j] = s_all[n, j, 23] + e_t[n, j]
                      nc.vector.tensor_tensor(
                          h_sl,
                          _bview(s_all[:, K - 1:K], [[KK, NT], [K, K]]),
                          e_sl, op=Alu.add)
                  nc.sync.dma_start(
                      _dview(adram, ch * CK,
                             [[Tc * K, P], [P * Tc * K, NT], [1, CK]]),
                      hist[:, :].rearrange("p (n c) -> p n c", c=CK))
                  prev_hist = hist
              fctx.close()
            tc.strict_bb_all_engine_barrier()

            # ---------------- backward (traceback) ----------------
            if phases in ("both", "bwd"):
              bctx = ctx.enter_context(ExitStack())
              ta = bctx.enter_context(tc.tile_pool(name="ta", bufs=2))
              to = bctx.enter_context(tc.tile_pool(name="to", bufs=2))
              tsm = bctx.enter_context(tc.tile_pool(name="tsm", bufs=3))
              th = bctx.enter_context(tc.tile_pool(name="th", bufs=3))
              tps = bctx.enter_context(tc.tile_pool(name="tps", bufs=2,
                                                    space="PSUM"))
              h_list = [None] * len(groups)
              for ch in range(NCH - 1, -1, -1):
                  ach = ta.tile([P, NT * CK], f32, tag="ach")
                  nc.sync.dma_start(
                      ach[:, :].rearrange("p (n c) -> p n c", c=CK),
                      _dview(adram, ch * CK,
                             [[Tc * K, P], [P * Tc * K, NT], [1, CK]]))
                  och = to.tile([P, NT * CK], f32, tag="och")
                  for t_c in range(CH - 1, -1, -1):
                      t = ch * CH + t_c
                      for gi, (g0, gn) in enumerate(groups):
                          GW = gn * K
                          a_sl = _bview(
                              ach[:, g0 * CK + t_c * K:g0 * CK + t_c * K + 1],
                              [[CK, gn], [1, K]])
                          r_t = tsm.tile([P, GW], f32, tag=f"r{gi}")
                          r_v = _bview(r_t[:, 0:1], [[K, gn], [1, K]])
                          if t == Tc - 1:
                              nc.vector._custom_dve(SEG_MAX1, out=r_v,
                                                    in0=a_sl)
                          else:
                              # gather g[p, n, i] = T[i, tag_{t+1}[p, n]]
                              htp = tps.tile([GW, P], f32, tag=f"htp{gi}")
                              nc.tensor.transpose(htp[:, :], h_list[gi],
                                                  idn[:, :])
                              hts = tsm.tile([GW, P], f32, tag=f"hts{gi}")
                              nc.vector.tensor_copy(hts[:, :], htp[:, :])
                              gp = tps.tile([P, GW], f32, tag=f"gp{gi}")
                              nc.tensor.matmul(gp[:, :], hts[:, :],
                                               bd[0:GW, 0:GW],
                                               start=True, stop=True)
                              nc.vector._custom_dve(
                                  SEG_MAXPLUS, out=r_v,
                                  in0=_bview(gp[:, 0:1], [[K, gn], [1, K]]),
                                  in1=a_sl)
                          h_new = th.tile([P, GW], f32, tag=f"h{gi}")
                          nc.vector._custom_dve(
                              SEG_ARGMAX_ONEHOT,
                              out=_bview(h_new[:, 0:1], [[K, gn], [1, K]]),
                              in0=r_v,
                              in1=_bview(r_t[:, K - 1:K], [[K, gn], [0, K]]),
                          )
                          h_list[gi] = h_new[:, :]
                          nc.gpsimd.tensor_copy(
                              _bview(och[:, g0 * CK + t_c * K:
                                         g0 * CK + t_c * K + 1],
                                     [[CK, gn], [1, K]]),
                              _bview(h_new[:, 0:1], [[K, gn], [1, K]]))
                  nc.sync.dma_start(
                      _dview(outp, ch * CK,
                             [[Tc * K, P], [P * Tc * K, NT], [1, CK]]),
                      och[:, :].rearrange("p (n c) -> p n c", c=CK))
              bctx.close()
    if split_waits:
        _split_excess_waits(nc)
    from concourse.library_overlay import lower_extended_insts
    lower_extended_insts(nc)
    return nc


def make_aux(transitions, BLc):
    """Host-side constant tensors derived from the transitions matrix."""
    NT = BLc // P
    groups = []
    s0 = 0
    while s0 < NT:
        g = min(4, NT - s0)
        groups.append((s0, g))
        s0 += g
    gn = max(g for _, g in groups)
    GK = gn * K
    Tm = np.asarray(transitions, dtype=np.float32)
    tb = np.ascontiguousarray(
        np.broadcast_to(Tm.T.reshape(1, K * K), (P, K * K))).astype(np.float32)
    bdm = np.zeros((GK, GK), np.float32)
    for g in range(gn):
        bdm[g * K:(g + 1) * K, g * K:(g + 1) * K] = Tm.T
    idn = np.eye(P, dtype=np.float32)
    return {"tbcast": tb, "blockdiag": bdm, "idn": idn}


def run(inputs, transitions, trace=False, **spmd_kwargs):
    from concourse.bass_utils import run_bass_kernel_spmd

    key = (BL, T)
    if key not in _prog_cache:
        _prog_cache[key] = build_program(BL, T, CH=32)
    nc = _prog_cache[key]

    inputs = np.asarray(inputs, dtype=np.float32)
    aux = make_aux(transitions, BL)
    in_maps = [
        {"inp": np.ascontiguousarray(inputs[c * BL:(c + 1) * BL]), **aux}
        for c in range(NCORES)
    ]
    res = run_bass_kernel_spmd(nc, in_maps, core_ids=list(range(NCORES)),
                               trace=trace, **spmd_kwargs)
    out = np.concatenate([r["outp"] for r in res.results], axis=0)
    return np.ascontiguousarray(out, dtype=np.float32), res


def kernel(inputs, transitions):
    out, _ = run(inputs, transitions)
    return out


# revision 3
# speedup vs baseline: 1.8420x; 1.0037x over previous
"""Linear-chain CRF Viterbi decode on Trainium2 (Bass/Tile), 8-core data
parallel — fused custom-DVE implementation.

Algorithm (bitwise-exact match to the f32 jax reference):
  forward:  alpha_0 = emit_0;  alpha_t[j] = max_i(alpha_{t-1}[i] + T[i,j]) + emit_t[j]
            Per batch tile, ONE custom DVE op (SEG_MAXPLUS) streams
            T^T[(j,i)] + alpha[i] and keeps a running max that RESETS at
            each j-segment boundary — the per-j max lands at i=23.  A single
            192-wide tensor_tensor adds emissions.  All alphas spill to a
            DRAM scratch buffer (DMA-hidden).
  backward: tag_T = first-argmax(alpha_T);
            tag_t = first-argmax_i(alpha_t[i] + T[i, tag_{t+1}]).
            The T[:, tag] column is gathered with a one-hot PE matmul
            (exact: products with 0.0/1.0).  SEG_MAXPLUS computes the
            running max r of (gather + alpha); SEG_ARGMAX_ONEHOT emits the
            one-hot of the FIRST position where r equals the segment max
            (ties resolve to the lowest index, matching jnp.argmax).
            Two independent 4-tile chains interleave to hide latency.
  output:   one-hot rows written as f32.

Custom DVE ops are registered at import: plain-Python uop programs shipped
in the per-NEFF DVE table (dve_table_for_ops); the segment-reset semantics
add one FSM step-state firing on SUB_DIM_DONE.  Excess semaphore waits ride
EventSemaphore carriers (2 waits each, sequencer-level).

Sharding: pure batch data-parallelism, 8192 rows -> 8 cores x 1024 rows.
"""

import numpy as np
from contextlib import ExitStack

B = 8192
T = 512
K = 24
NCORES = 8
BL = B // NCORES          # rows per core
P = 128                   # partitions

_prog_cache = {}

# --------------------------------------------------------------------------
# Custom DVE ops: segmented (per-24) folds with reset at segment boundaries.
# --------------------------------------------------------------------------

_OPS_REGISTERED = {}


def _register_ops():
    if _OPS_REGISTERED:
        return _OPS_REGISTERED

    import concourse.dve_spec as DS
    import concourse.dve_ops as DO
    from concourse.dve_spec import Spec, Src0, Src1, One, scan, eq, AluOp
    from concourse.dve_uop import DveOpSpec, Trigger

    def _lower_segreset(spec, ver):
        """lower() plus a step-state that re-seeds every scan from its init
        at SUB_DIM_DONE (segment boundary) for exactly one element."""
        n_lanes, n_stages = DS.N_LANES[ver], DS.N_STAGES[ver]
        DS._validate_body(spec, ver)
        spec = DS._hoist_stream_invariant_ops(spec)
        scans = DS._collect(spec.body, DS.Scan)
        latches = DS._collect(spec.body, DS.Latch)
        assert scans and not latches and spec.accum is None
        placement = DS._build_placement(spec, scans, n_stages, n_lanes)
        states = list(DS._build_state_machine(spec, scans, latches, placement))
        assert len(states) == 2  # [seed, steady]
        step_ov = {}
        for s in scans:
            d = placement.node_stage[s]
            step_ov[d] = DS._Stage(s.op, DS._scan_init(s), s.expr)
        body_lvs = DS._body_scan_leaves(spec)
        consume = (Src0 in body_lvs, Src1 in body_lvs)
        st = states[1]
        states[1] = DS._State(
            placement=st.placement, consume=st.consume, overrides=st.overrides,
            trigger=(Trigger.SRC_TENSOR_DONE, Trigger.SUB_DIM_DONE,
                     Trigger.NONE),
            next=(0, 2, 0))
        states.append(DS._State(
            placement=placement, consume=consume, overrides=step_ov,
            trigger=(Trigger.SRC_TENSOR_DONE, Trigger.SUB_DIM_DONE,
                     Trigger.COUNT),
            next=(0, 2, 1), repeat=1))
        out = [DS._assemble(s) for s in states]
        for u in out:
            u.validate(ver)
        return out

    class SegDveOp(DO.DveOp):
        def compile(self, ver):
            key = (self.name, ver)
            if (r := DO._COMPILE_CACHE.get(key)) is not None:
                return r
            result = DveOpSpec(
                name=self.name,
                opcode=DO.get_dve_sub_opcode(self.name),
                uops=_lower_segreset(self.spec, ver),
                rd1_en=DS._has_src1(self.spec),
            )
            DO._COMPILE_CACHE[key] = result
            return result

    def _ref_seg_maxplus(in0, in1, c0, c1, c2):
        s = (in0 + in1).astype(np.float32)
        out = np.empty_like(s)
        run = None
        for k in range(s.shape[-1]):
            run = s[..., k] if run is None else np.maximum(run, s[..., k])
            out[..., k] = run
        return out

    def _ref_seg_max1(in0, c0, c1, c2):
        return _ref_seg_maxplus(in0, np.zeros_like(in0), c0, c1, c2)

    def _ref_seg_argmax_onehot(in0, in1, c0, c1, c2):
        e = (in0 == in1).astype(np.float32)
        c = np.cumsum(e, axis=-1)
        return (e * (c == 1.0)).astype(np.float32)

    def _mk(name, spec):
        existing = [op for op in DO.OPS if op.name == name]
        if existing:
            return existing[0]
        op = SegDveOp(name=name, spec=spec, subdim=True, uops_sha={})
        DO.OPS.append(op)
        row = DO._CUSTOM_DVE_ROW_BASE + len(DO.OPS) - 1
        assert row < 0x20, "custom-DVE opcode row overflow"
        DO._SUB_OPCODE_FOR_NAME[name] = row
        return op

    _e = eq(Src0, Src1)
    _OPS_REGISTERED.update(
        SEG_MAXPLUS=_mk("SEG_MAXPLUS",
                        Spec(body=scan(AluOp.MAX, Src0 + Src1),
                             reference=_ref_seg_maxplus)),
        SEG_MAX1=_mk("SEG_MAX1",
                     Spec(body=scan(AluOp.MAX, Src0),
                          reference=_ref_seg_max1)),
        SEG_ARGMAX_ONEHOT=_mk("SEG_ARGMAX_ONEHOT",
                              Spec(body=_e * eq(scan(AluOp.ADD, _e), One),
                                   reference=_ref_seg_argmax_onehot)),
    )
    return _OPS_REGISTERED


# --------------------------------------------------------------------------
# AP helpers
# --------------------------------------------------------------------------

def _bview(sl, dims):
    """AP with custom free dims (incl. step-0 broadcast), keeping the slice's
    offset and partition pair."""
    from concourse.ap import AP
    return AP(sl.tensor, sl.offset, [list(sl.ap[0])] + [list(d) for d in dims])


def _dview(ap, offset, dims):
    """Arbitrary strided view of a DRAM tensor ([[step,count],...], offset)."""
    from concourse.ap import AP
    return AP(ap.tensor, offset, [list(d) for d in dims])


def _split_excess_waits(nc):
    """Walrus allows at most one semaphore wait per instruction (two on
    InstEventSemaphore).  Move excess waits onto EventSemaphore carriers
    (sequencer-level, no pipeline flush) inserted immediately before the
    instruction — engine streams execute in order, so this is semantically
    identical."""
    from concourse import mybir
    ctr = 0
    for f in nc.m.functions:
        for blk in f.blocks:
            changed = False
            out = []
            for ins in blk.instructions:
                si = ins.sync_info
                if si is not None and len(si.on_wait) > 1:
                    excess = list(si.on_wait[:-1])
                    keep = si.on_wait[-1]
                    while excess:
                        pair, excess = excess[:2], excess[2:]
                        ev = mybir.InstEventSemaphore(
                            name=f"EW-{ctr}", ins=[], outs=[])
                        ctr += 1
                        ev.engine = ins.engine
                        ev.sync_info = mybir.SyncInfo(on_wait=pair,
                                                      on_update=[])
                        out.append(ev)
                    ins.sync_info = mybir.SyncInfo(
                        on_wait=[keep], on_update=list(si.on_update))
                    changed = True
                out.append(ins)
            if changed:
                blk.instructions = out
    return ctr


# --------------------------------------------------------------------------
# Program builder
# --------------------------------------------------------------------------

def build_program(BLc, Tc, CH, split_waits=True, repeat=1, phases="both"):
    import concourse.bass as bass
    import concourse.tile as tile
    from concourse import mybir

    ops = _register_ops()
    SEG_MAXPLUS = ops["SEG_MAXPLUS"]
    SEG_MAX1 = ops["SEG_MAX1"]
    SEG_ARGMAX_ONEHOT = ops["SEG_ARGMAX_ONEHOT"]

    f32 = mybir.dt.float32
    Alu = mybir.AluOpType

    NT = BLc // P             # batch tiles per core
    NCH = Tc // CH            # time chunks
    CK = CH * K               # free elems per (tile, chunk)
    KK = K * K
    groups = []
    s0 = 0
    while s0 < NT:
        g = min(4, NT - s0)
        groups.append((s0, g))
        s0 += g
    GK = max(g for _, g in groups) * K

    nc = bass.Bass("TRN2", target_bir_lowering=False, debug=False)

    inp = nc.dram_tensor("inp", [BLc, Tc, K], f32, kind="ExternalInput").ap()
    tb_d = nc.dram_tensor("tbcast", [P, KK], f32, kind="ExternalInput").ap()
    bd_d = nc.dram_tensor("blockdiag", [GK, GK], f32, kind="ExternalInput").ap()
    idn_d = nc.dram_tensor("idn", [P, P], f32, kind="ExternalInput").ap()
    outp = nc.dram_tensor("outp", [BLc, Tc, K], f32, kind="ExternalOutput").ap()
    adram = nc.dram_tensor("alpha_scr", [NT, P, Tc, K], f32, kind="Internal").ap()

    with tile.TileContext(nc) as tc, ExitStack() as ctx:
        const = ctx.enter_context(tc.tile_pool(name="const", bufs=1))

        tb = const.tile([P, KK], f32)
        nc.sync.dma_start(tb[:, :], tb_d)
        bd = const.tile([GK, GK], f32)
        nc.sync.dma_start(bd[:, :], bd_d)
        idn = const.tile([P, P], f32)
        nc.sync.dma_start(idn[:, :], idn_d)

        for _rep in range(repeat):
            if _rep:
                tc.strict_bb_all_engine_barrier()
            # ---------------- forward ----------------
            if phases in ("both", "fwd"):
              fctx = ctx.enter_context(ExitStack())
              femis = fctx.enter_context(tc.tile_pool(name="femis", bufs=2))
              fhist = fctx.enter_context(tc.tile_pool(name="fhist", bufs=2))
              fs = fctx.enter_context(tc.tile_pool(name="fs", bufs=2))
              prev_hist = None
              for ch in range(NCH):
                  emis = femis.tile([P, NT * CK], f32, tag="emis")
                  nc.sync.dma_start(
                      emis[:, :].rearrange("p (n c) -> p n c", c=CK),
                      _dview(inp, ch * CK,
                             [[Tc * K, P], [P * Tc * K, NT], [1, CK]]))
                  hist = fhist.tile([P, NT * CK], f32, tag="hist")
                  for t_c in range(CH):
                      t = ch * CH + t_c
                      h_sl = _bview(hist[:, t_c * K:t_c * K + 1], [[CK, NT], [1, K]])
                      e_sl = _bview(emis[:, t_c * K:t_c * K + 1], [[CK, NT], [1, K]])
                      if t == 0:
                          nc.vector.tensor_copy(h_sl, e_sl)
                          continue
                      if t_c == 0:
                          a_base, a_off = prev_hist, (CH - 1) * K
                      else:
                          a_base, a_off = hist, (t_c - 1) * K
                      s_all = fs.tile([P, NT * KK], f32, tag="sall")
                      for n in range(NT):
                          nc.vector._custom_dve(
                              SEG_MAXPLUS,
                              out=_bview(s_all[:, n * KK:n * KK + 1],
                                         [[K, K], [1, K]]),
                              in0=_bview(tb[:, 0:1], [[K, K], [1, K]]),
                              in1=_bview(a_base[:, n * CK + a_off:
                                                n * CK + a_off + 1],
                                         [[0, K], [1, K]]),
                          )
                      # alpha_t[n,j] = s_all[n, j, 23] + e_t[n, j]
                      nc.vector.tensor_tensor(
                          h_sl,
                          _bview(s_all[:, K - 1:K], [[KK, NT], [K, K]]),
                          e_sl, op=Alu.add)
                  nc.sync.dma_start(
                      _dview(adram, ch * CK,
                             [[Tc * K, P], [P * Tc * K, NT], [1, CK]]),
                      hist[:, :].rearrange("p (n c) -> p n c", c=CK))
                  prev_hist = hist
              fctx.close()
            tc.strict_bb_all_engine_barrier()

            # ---------------- backward (traceback) ----------------
            if phases in ("both", "bwd"):
              bctx = ctx.enter_context(ExitStack())
              ta = bctx.enter_context(tc.tile_pool(name="ta", bufs=2))
              to = bctx.enter_context(tc.tile_pool(name="to", bufs=2))
              tsm = bctx.enter_context(tc.tile_pool(name="tsm", bufs=3))
              th = bctx.enter_context(tc.tile_pool(name="th", bufs=3))
              tps = bctx.enter_context(tc.tile_pool(name="tps", bufs=2,
                                                    space="PSUM"))
              h_list = [None] * len(groups)
              for ch in range(NCH - 1, -1, -1):
                  ach = ta.tile([P, NT * CK], f32, tag="ach")
                  nc.sync.dma_start(
                      ach[:, :].rearrange("p (n c) -> p n c", c=CK),
                      _dview(adram, ch * CK,
                             [[Tc * K, P], [P * Tc * K, NT], [1, CK]]))
                  och = to.tile([P, NT * CK], f32, tag="och")
                  for t_c in range(CH - 1, -1, -1):
                      t = ch * CH + t_c
                      for gi, (g0, gn) in enumerate(groups):
                          GW = gn * K
                          a_sl = _bview(
                              ach[:, g0 * CK + t_c * K:g0 * CK + t_c * K + 1],
                              [[CK, gn], [1, K]])
                          r_t = tsm.tile([P, GW], f32, tag=f"r{gi}")
                          r_v = _bview(r_t[:, 0:1], [[K, gn], [1, K]])
                          if t == Tc - 1:
                              nc.vector._custom_dve(SEG_MAX1, out=r_v,
                                                    in0=a_sl)
                          else:
                              # gather g[p, n, i] = T[i, tag_{t+1}[p, n]]
                              htp = tps.tile([GW, P], f32, tag=f"htp{gi}")
                              nc.tensor.transpose(htp[:, :], h_list[gi],
                                                  idn[:, :])
                              hts = tsm.tile([GW, P], f32, tag=f"hts{gi}")
                              nc.vector.tensor_copy(hts[:, :], htp[:, :])
                              gp = tps.tile([P, GW], f32, tag=f"gp{gi}")
                              nc.tensor.matmul(gp[:, :], hts[:, :],
                                               bd[0:GW, 0:GW],
                                               start=True, stop=True)
                              nc.vector._custom_dve(
                                  SEG_MAXPLUS, out=r_v,
                                  in0=_bview(gp[:, 0:1], [[K, gn], [1, K]]),
                                  in1=a_sl)
                          h_new = th.tile([P, GW], f32, tag=f"h{gi}")
                          nc.vector._custom_dve(
                              SEG_ARGMAX_ONEHOT,
                              out=_bview(h_new[:, 0:1], [[K, gn], [1, K]]),
                              in0=r_v,
                              in1=_bview(r_t[:, K - 1:K], [[K, gn], [0, K]]),
                          )
                          h_list[gi] = h_new[:, :]
                          nc.gpsimd.tensor_copy(
                              _bview(och[:, g0 * CK + t_c * K:
                                         g0 * CK + t_c * K + 1],
                                     [[CK, gn], [1, K]]),
                              _bview(h_new[:, 0:1], [[K, gn], [1, K]]))
                  nc.sync.dma_start(
                      _dview(outp, ch * CK,
                             [[Tc * K, P], [P * Tc * K, NT], [1, CK]]),
                      och[:, :].rearrange("p (n c) -> p n c", c=CK))
              bctx.close()
    if split_waits:
        _split_excess_waits(nc)
    from concourse.library_overlay import lower_extended_insts
    lower_extended_insts(nc)
    return nc


def make_aux(transitions, BLc):
    """Host-side constant tensors derived from the transitions matrix."""
    NT = BLc // P
    groups = []
    s0 = 0
    while s0 < NT:
        g = min(4, NT - s0)
        groups.append((s0, g))
        s0 += g
    gn = max(g for _, g in groups)
    GK = gn * K
    Tm = np.asarray(transitions, dtype=np.float32)
    tb = np.ascontiguousarray(
        np.broadcast_to(Tm.T.reshape(1, K * K), (P, K * K))).astype(np.float32)
    bdm = np.zeros((GK, GK), np.float32)
    for g in range(gn):
        bdm[g * K:(g + 1) * K, g * K:(g + 1) * K] = Tm.T
    idn = np.eye(P, dtype=np.float32)
    return {"tbcast": tb, "blockdiag": bdm, "idn": idn}


def run(inputs, transitions, trace=False, **spmd_kwargs):
    from concourse.bass_utils import run_bass_kernel_spmd

    key = (BL, T)
    if key not in _prog_cache:
        _prog_cache[key] = build_program(BL, T, CH=16)
    nc = _prog_cache[key]

    inputs = np.asarray(inputs, dtype=np.float32)
    aux = make_aux(transitions, BL)
    in_maps = [
        {"inp": np.ascontiguousarray(inputs[c * BL:(c + 1) * BL]), **aux}
        for c in range(NCORES)
    ]
    res = run_bass_kernel_spmd(nc, in_maps, core_ids=list(range(NCORES)),
                               trace=trace, **spmd_kwargs)
    out = np.concatenate([r["outp"] for r in res.results], axis=0)
    return np.ascontiguousarray(out, dtype=np.float32), res


def kernel(inputs, transitions):
    out, _ = run(inputs, transitions)
    return out
